# revision 44
# baseline (speedup 1.0000x reference)
"""Multi-headed causal attention (B=2, S=2048, D=1024, H=16, DK=DV=64) on 8
Trainium2 NeuronCores.

Sharding: HEAD-parallel, zero-communication. Core c owns heads {2c, 2c+1}
for BOTH batches: it projects K/Q/V for its two heads only (no redundant
FLOPs), runs their full causal attention, and output-projects ALL 4096
queries against its two heads' Wo rows, emitting a bf16 PARTIAL output.
The host sums the 8 partials and adds the output bias -- collectives in
this stack act as full program barriers (~30us each), so finishing on
the host is cheaper. Head-sharding makes the fine-causal tile structure
(only kt<=t score tiles) IDENTICAL on every core, as SPMD requires.

All matmul operands are bf16 (full PE rate at any free-dim). The PE is
HAM-clock-gated: idle gaps drop it to 1.2GHz, so emission keeps the PE
dense: b1's projections fill attention(b0) banks, b0's output projection
fills attention(b1) banks (deadline+pace scheduler). Later attention
phases exp over two PSUM banks per ACT instruction; causal masks run on
the idle Pool engine. Softmax: no max-subtraction; denominators via an
all-ones V column; reciprocal = fast-approx DVE op on the ACT-shifted
denominator row, replicated by a K=1 f32r matmul, applied in-place.
"""

import numpy as np

B, S, D, H, DK = 2, 2048, 1024, 16, 64
NCORES = 8
NT = S // 128
NBANKS = 34

_BUILT = {}


def _build_nc(general_mask):
    import concourse.bacc as bacc
    import concourse.mybir as mybir
    from concourse import tile
    from contextlib import ExitStack

    f32 = mybir.dt.float32
    f32r = mybir.dt.float32r
    bf16 = mybir.dt.bfloat16
    AF = mybir.ActivationFunctionType
    ALU = mybir.AluOpType

    nc = bacc.Bacc("TRN2", target_bir_lowering=False, debug=False,
                   num_devices=NCORES)

    xk_t = nc.declare_dram_parameter("xk_t", [2 * D, S], bf16, isOutput=False)
    xq_t = nc.declare_dram_parameter("xq_t", [2 * D, S], bf16, isOutput=False)
    xv_t = nc.declare_dram_parameter("xv_t", [2 * D, S], bf16, isOutput=False)
    wk_h = nc.declare_dram_parameter("wk_h", [D, 128], bf16, isOutput=False)
    wq_h = nc.declare_dram_parameter("wq_h", [D, 128], bf16, isOutput=False)
    wv_p = nc.declare_dram_parameter("wv_p", [D, 130], bf16, isOutput=False)
    wo_h = nc.declare_dram_parameter("wo_h", [128, D], bf16, isOutput=False)
    bk_h = nc.declare_dram_parameter("bk_h", [128, 1], f32, isOutput=False)
    bq_h = nc.declare_dram_parameter("bq_h", [128, 1], f32, isOutput=False)
    bv_p = nc.declare_dram_parameter("bv_p", [1, 130], f32, isOutput=False)
    ones1 = nc.declare_dram_parameter("ones1", [1, 128], f32r, isOutput=False)
    trimask = nc.declare_dram_parameter("trimask", [128, 128], bf16,
                                        isOutput=False)
    if general_mask:
        maskb = nc.declare_dram_parameter(
            "maskb", [2 * NBANKS * 128, 512], bf16, isOutput=False)
    outp = nc.declare_dram_parameter("outp", [2 * S, D], bf16, isOutput=True)

    with tile.TileContext(nc) as tc:
        with ExitStack() as ctx:
            persist = ctx.enter_context(tc.tile_pool(name="persist", bufs=1))

            wk_sb = [persist.tile([128, 128], bf16, name=f"wk{i}",
                                  tag=f"wk{i}") for i in range(8)]
            wq_sb = [persist.tile([128, 128], bf16, name=f"wq{i}",
                                  tag=f"wq{i}") for i in range(8)]
            wv_sb = [persist.tile([128, 130], bf16, name=f"wv{i}",
                                  tag=f"wv{i}") for i in range(8)]
            wo_sb = persist.tile([128, D], bf16, name="wo", tag="wo")
            bk_sb = persist.tile([128, 1], f32, name="bk", tag="bk")
            bq_sb = persist.tile([128, 1], f32, name="bq", tag="bq")
            tri_sb = persist.tile([128, 128], bf16, name="tri", tag="tri")
            bvr_sb = persist.tile([1, 130], f32, name="bvr", tag="bvr")
            ones_sb = persist.tile([1, 128], f32r, name="ones", tag="ones")
            bv_rep = persist.tile([128, 130], f32, name="bvrep", tag="bvrep")
            kT = [persist.tile([128, S], bf16, name=f"kT{b}", tag=f"kT{b}")
                  for b in range(B)]
            qT = [persist.tile([128, S], bf16, name=f"qT{b}", tag=f"qT{b}")
                  for b in range(B)]
            v_sb = [[persist.tile([128, 130], bf16, name=f"v{b}_{st}",
                                  tag=f"v{b}_{st}") for st in range(NT)]
                    for b in range(B)]
            navTh = [[persist.tile([64, S], bf16, name=f"nav{b}_{hh}",
                                   tag=f"nav{b}_{hh}") for hh in range(2)]
                     for b in range(B)]
            navT2 = [persist.tile([128, S], bf16, name=f"nv2_{b}",
                                  tag=f"nv2_{b}") for b in range(B)]

            xs = ctx.enter_context(tc.tile_pool(name="xs", bufs=2))
            amp = ctx.enter_context(tc.tile_pool(name="amp", bufs=3))
            nrm = ctx.enter_context(tc.tile_pool(name="nrm", bufs=2))
            fop = ctx.enter_context(tc.tile_pool(name="fop", bufs=3))
            avp = ctx.enter_context(tc.tile_pool(name="avp", bufs=2,
                                                 space="PSUM"))
            repp = ctx.enter_context(tc.tile_pool(name="repp", bufs=1,
                                                  space="PSUM"))
            if general_mask:
                mbp = ctx.enter_context(tc.tile_pool(name="mbp", bufs=4))

            # ---- P0: K-path loads first so the PE starts ASAP ----
            for i in range(8):
                nc.sync.dma_start(wk_sb[i][:], wk_h[128 * i:128 * (i + 1), :])
            nc.sync.dma_start(bk_sb[:], bk_h[:])

            def load_x_halves(param, b):
                tiles = [xs.tile([128, S], bf16, name=f"x{kp}", tag=f"x{kp}")
                         for kp in range(8)]
                for h in range(2):
                    for kp in range(8):
                        eng = nc.sync if kp % 2 == 0 else nc.scalar
                        eng.dma_start(
                            tiles[kp][:, 1024 * h:1024 * (h + 1)],
                            param[D * b + 128 * kp:D * b + 128 * (kp + 1),
                                  1024 * h:1024 * (h + 1)])
                return tiles

            pools = {}

            def proj_kq_unit(x, w_sb, bias_sb, dst, sc):
                ps = pools["pp"].tile([128, 512], f32, name="pp", tag="pp")
                for kp in range(8):
                    nc.tensor.matmul(ps[:], w_sb[kp][:],
                                     x[kp][:, 512 * sc:512 * (sc + 1)],
                                     start=(kp == 0), stop=(kp == 7))
                nc.vector.tensor_scalar_add(
                    dst[:, 512 * sc:512 * (sc + 1)], ps[:], bias_sb[:])

            def proj_v_unit(x, b, st):
                ps = pools["pp"].tile([128, 512], f32, name="pp", tag="pp")
                for kp in range(8):
                    nc.tensor.matmul(ps[:, 0:130],
                                     x[kp][:, 128 * st:128 * (st + 1)],
                                     wv_sb[kp][:],
                                     start=(kp == 0), stop=(kp == 7))
                nc.vector.tensor_tensor(v_sb[b][st][:], ps[:, 0:130],
                                        bv_rep[:], ALU.add)

            def op_unit(b, qt, oc, u):
                # partial output projection: this core's 2 heads only,
                # K=128, one matmul; host sums partials across cores
                ps = pools["scp2"].tile([128, 1024], f32, name="sc",
                                        tag="sc")
                nc.tensor.matmul(ps[:, 0:512],
                                 navT2[b][:, 128 * qt:128 * (qt + 1)],
                                 wo_sb[:, 512 * oc:512 * (oc + 1)],
                                 start=True, stop=True)
                fo = fop.tile([128, 512], bf16, name="fo", tag="fo")
                nc.vector.tensor_copy(fo[:], ps[:, 0:512])
                nc.sync.dma_start(
                    outp[S * b + 128 * qt:S * b + 128 * (qt + 1),
                         512 * oc:512 * (oc + 1)], fo[:])

            def norm_stage1(b, hh, av, T, dn=None):
                dg0 = nrm.tile([1, 512], f32, name="dg0", tag="dg0")
                if dn is None:
                    # denominator row on PSUM partition 64: only ACT can
                    # shift partitions
                    nc.scalar.copy(dg0[:], av[64:65, :])
                else:
                    # wide phase: denominators accumulated at partition 0
                    # in their own PSUM row -> plain aligned DVE copy,
                    # keeping ACT exp-only
                    nc.vector.tensor_copy(dg0[:], dn[0:1, :])
                dg = nrm.tile([1, 512], f32, name="dg", tag="dg")
                nc.vector.reciprocal_approx_fast(dg[:], dg0[:])
                dgr = nrm.tile([1, 512], f32r, name="dgr", tag="dgr")
                nc.vector.tensor_scalar_add(dgr[:], dg[:], 0.0)
                nc.vector.tensor_copy(
                    navTh[b][hh][:, 512 * T:512 * (T + 1)], av[0:64, :])
                return dgr

            def norm_stage2(b, hh, dgr, T):
                rp = repp.tile([64, 512], f32, name="rp", tag="rp")
                nc.tensor.matmul(rp[:], ones_sb[:, 0:64], dgr[:],
                                 start=True, stop=True)
                sl = navTh[b][hh][:, 512 * T:512 * (T + 1)]
                nc.vector.tensor_tensor(sl, sl, rp[:], ALU.mult)
                nc.sync.dma_start(
                    navT2[b][64 * hh:64 * (hh + 1),
                             512 * T:512 * (T + 1)], sl)

            stream = [(t, kt) for t in range(NT) for kt in range(t + 1)]
            banks = [stream[i:i + 4] for i in range(0, len(stream), 4)]

            fill_units = []
            fill_state = {"idx": 0}

            def pump(g, g_lo=0, g_hi=0):
                # strictly deadline-driven: deadlines both spread the filler
                # and guarantee producers are emitted before consumers
                n = len(fill_units)
                while fill_state["idx"] < n:
                    i = fill_state["idx"]
                    if fill_units[i][0] > g:
                        break
                    fill_units[i][1]()
                    fill_state["idx"] += 1

            def attention(b, hh, g0, g_lo, g_hi, wide=False):
                r0 = 64 * hh
                step = 2 if wide else 1
                av = None
                pending = None
                for bi in range(0, NBANKS, step):
                    pump(g0 + bi, g_lo, g_hi)
                    flat = [tk for bk in banks[bi:bi + step] for tk in bk]
                    width = 128 * len(flat)
                    sc = pools["scp2" if wide else "scp"].tile(
                        [128, 1024 if wide else 512], f32, name="sc",
                        tag="sc")
                    for s, (t, kt) in enumerate(flat):
                        nc.tensor.matmul(
                            sc[:, 128 * s:128 * (s + 1)],
                            kT[b][r0:r0 + 64, 128 * kt:128 * (kt + 1)],
                            qT[b][r0:r0 + 64, 128 * t:128 * (t + 1)],
                            start=True, stop=True)
                    am = amp.tile([128, 1024 if wide else 512], bf16,
                                  name="am", tag="am2" if wide else "am")
                    nc.scalar.activation(am[:, 0:width], sc[:, 0:width],
                                         AF.Exp, scale=0.125)
                    if general_mask:
                        for k2 in range((len(flat) + 3) // 4):
                            mb = mbp.tile([128, 512], bf16, name="mb",
                                          tag="mb")
                            r = (b * NBANKS + bi + k2) * 128
                            nc.sync.dma_start(mb[:], maskb[r:r + 128, :])
                            nc.vector.tensor_tensor(
                                am[:, 512 * k2:512 * (k2 + 1)],
                                am[:, 512 * k2:512 * (k2 + 1)],
                                mb[:], ALU.mult)
                    else:
                        for s, (t, kt) in enumerate(flat):
                            if t == kt:
                                nc.gpsimd.tensor_tensor(
                                    am[:, 128 * s:128 * (s + 1)],
                                    am[:, 128 * s:128 * (s + 1)],
                                    tri_sb[:], ALU.mult)
                    for s, (t, kt) in enumerate(flat):
                        if kt == 0 and t % 4 == 0:
                            av = avp.tile([65, 512], f32, name="av",
                                          tag="av")
                            if wide:
                                dn = pools["dnp"].tile(
                                    [1, 512], f32, name="dn", tag="dn")
                        nc.tensor.matmul(
                            av[:, 128 * (t % 4):128 * (t % 4 + 1)],
                            v_sb[b][kt][:, 65 * hh:65 * (hh + 1)],
                            am[:, 128 * s:128 * (s + 1)],
                            start=(kt == 0), stop=(kt == t))
                        if wide:
                            nc.tensor.matmul(
                                dn[:, 128 * (t % 4):128 * (t % 4 + 1)],
                                v_sb[b][kt][:, 65 * hh + 64:65 * hh + 65],
                                am[:, 128 * s:128 * (s + 1)],
                                start=(kt == 0), stop=(kt == t))
                        if kt == t and t % 4 == 3:
                            if pending is not None:
                                norm_stage2(b, hh, pending[0], pending[1])
                            dgr = norm_stage1(b, hh, av, t // 4,
                                              dn if wide else None)
                            pending = (dgr, t // 4)
                if pending is not None:
                    norm_stage2(b, hh, pending[0], pending[1])

            # ---- P0 continued + b0 K/Q projection ----
            xk0 = load_x_halves(xk_t, 0)
            for i in range(8):
                nc.scalar.dma_start(wq_sb[i][:],
                                    wq_h[128 * i:128 * (i + 1), :])
                nc.gpsimd.dma_start(wv_sb[i][:],
                                    wv_p[128 * i:128 * (i + 1), :])
            nc.gpsimd.dma_start(wo_sb[:], wo_h[:])
            nc.gpsimd.dma_start(bq_sb[:], bq_h[:])
            nc.gpsimd.dma_start(tri_sb[:], trimask[:])
            nc.gpsimd.dma_start(bvr_sb[:], bv_p[:])
            nc.gpsimd.dma_start(ones_sb[:], ones1[:])
            nc.gpsimd.partition_broadcast(bv_rep[:], bvr_sb[:])

            with tc.tile_pool(name="pp", bufs=2, space="PSUM") as pp, \
                 tc.tile_pool(name="scp", bufs=3, space="PSUM") as scp:
                pools["pp"] = pp
                pools["scp"] = scp

                for sc_i in range(4):
                    proj_kq_unit(xk0, wk_sb, bk_sb, kT[0], sc_i)
                xq0 = load_x_halves(xq_t, 0)
                for sc_i in range(4):
                    proj_kq_unit(xq0, wq_sb, bq_sb, qT[0], sc_i)
                xv0 = load_x_halves(xv_t, 0)

                def dl_diag(st):
                    return (st * (st + 3) // 2) // 4

                for st in range(NT):
                    fill_units.append(
                        (dl_diag(st),
                         (lambda st=st: proj_v_unit(xv0, 0, st))))
                # b1 projections spread over (0,0)+(0,1); consumers start
                # at g=68 so only the xs-ring order matters: k before q
                xk1_t = {}
                fill_units.append(
                    (36, lambda: xk1_t.update(x=load_x_halves(xk_t, 1))))
                for sc_i in range(4):
                    fill_units.append(
                        (38 + 2 * sc_i,
                         (lambda sc_i=sc_i: proj_kq_unit(
                             xk1_t["x"], wk_sb, bk_sb, kT[1], sc_i))))
                xv1_t = {}
                fill_units.append(
                    (46, lambda: xv1_t.update(x=load_x_halves(xv_t, 1))))
                for st in range(NT):
                    fill_units.append(
                        (47 + st,
                         (lambda st=st: proj_v_unit(xv1_t["x"], 1, st))))
                xq1_t = {}
                fill_units.append(
                    (48, lambda: xq1_t.update(x=load_x_halves(xq_t, 1))))
                for sc_i in range(4):
                    fill_units.append(
                        (50 + 4 * sc_i,
                         (lambda sc_i=sc_i: proj_kq_unit(
                             xq1_t["x"], wq_sb, bq_sb, qT[1], sc_i))))
                fill_units.sort(key=lambda u: u[0])

                attention(0, 0, 0, 0, 68)
                attention(0, 1, 34, 0, 68)
                pump(68, 0, 68)

            with tc.tile_pool(name="scp2", bufs=2, space="PSUM") as scp2, \
                 tc.tile_pool(name="dnp", bufs=1, space="PSUM") as dnp:
                pools["scp2"] = scp2
                pools["dnp"] = dnp

                # b0 output-projection partials fill attention(b1)
                fill_units.clear()
                fill_state["idx"] = 0
                for u in range(32):
                    qt, oc = u // 2, u % 2
                    fill_units.append(
                        (70 + 2 * u,
                         (lambda qt=qt, oc=oc, u=u: op_unit(0, qt, oc, u))))

                attention(1, 0, 68, 68, 136, wide=True)
                # b1 output projection: navT2[1] block T is complete once
                # both (1,0) and (1,1) have normalized it -> pump early
                # units during (1,1), drain the rest as the tail
                for u in range(32):
                    qt, oc = u // 2, u % 2
                    T = qt // 4
                    # navT2[1] block T's lagged stage2 is emitted by the end
                    # of block T+1 (~bank 8*(T+2) of (1,1)); T>=2 drains in
                    # the tail
                    dl = 137 if T >= 2 else (112 + 10 * T + (u % 8))
                    fill_units.append(
                        (dl,
                         (lambda qt=qt, oc=oc, u=u: op_unit(1, qt, oc, u))))
                attention(1, 1, 102, 68, 136, wide=True)
                pump(137)

    nc.compile()
    return nc


def kernel(V, K, Q, padding_mask, Wv_w, Wv_b, Wk_w, Wk_b, Wq_w, Wq_b,
           Wo_w, Wo_b):
    import ml_dtypes
    from concourse.bass_utils import run_bass_kernel_spmd
    bf = ml_dtypes.bfloat16

    V = np.asarray(V, np.float32)
    K = np.asarray(K, np.float32)
    Q = np.asarray(Q, np.float32)
    pm = np.asarray(padding_mask)
    Wv_w = np.asarray(Wv_w, np.float32)
    Wv_b = np.asarray(Wv_b, np.float32)
    Wk_w = np.asarray(Wk_w, np.float32)
    Wk_b = np.asarray(Wk_b, np.float32)
    Wq_w = np.asarray(Wq_w, np.float32)
    Wq_b = np.asarray(Wq_b, np.float32)
    Wo_w = np.asarray(Wo_w, np.float32)
    Wo_b = np.asarray(Wo_b, np.float32)

    general = not bool((pm != 0).all())
    key = "gen" if general else "fast"
    if key not in _BUILT:
        _BUILT[key] = _build_nc(general)
    nc = _BUILT[key]

    xk = np.concatenate(
        [np.ascontiguousarray(K[b].T) for b in range(B)], 0).astype(bf)
    xq = np.concatenate(
        [np.ascontiguousarray(Q[b].T) for b in range(B)], 0).astype(bf)
    xv = np.concatenate(
        [np.ascontiguousarray(V[b].T) for b in range(B)], 0).astype(bf)
    ones1 = np.ones((1, 128), np.float32)
    tri = (np.arange(128)[:, None] <= np.arange(128)[None, :])

    maskb_arr = None
    if general:
        stream = [(t, kt) for t in range(NT) for kt in range(t + 1)]
        bank_list = [stream[i:i + 4] for i in range(0, len(stream), 4)]
        maskb_arr = np.zeros((2 * NBANKS * 128, 512), np.float32)
        for b in range(B):
            keymask = (pm[b] != 0).astype(np.float32)
            for bi, bank in enumerate(bank_list):
                blk = np.zeros((128, 512), np.float32)
                for s, (t, kt) in enumerate(bank):
                    m = np.ones((128, 128), np.float32) if kt < t \
                        else tri.astype(np.float32)
                    blk[:, 128 * s:128 * (s + 1)] = (
                        m * keymask[128 * kt:128 * (kt + 1)][:, None])
                maskb_arr[(b * NBANKS + bi) * 128:
                          (b * NBANKS + bi + 1) * 128] = blk
        maskb_arr = maskb_arr.astype(bf)

    in_maps = []
    for c in range(NCORES):
        rows = slice(128 * c, 128 * (c + 1))
        wk_c = np.ascontiguousarray(Wk_w[rows].T).astype(bf)
        wq_c = np.ascontiguousarray(Wq_w[rows].T).astype(bf)
        wv_c = np.ascontiguousarray(Wv_w[rows].T)
        wv_pad = np.zeros((D, 130), np.float32)
        wv_pad[:, 0:64] = wv_c[:, 0:64]
        wv_pad[:, 65:129] = wv_c[:, 64:128]
        bv_pad = np.zeros((1, 130), np.float32)
        bv_pad[0, 0:64] = Wv_b[128 * c:128 * c + 64]
        bv_pad[0, 64] = 1.0
        bv_pad[0, 65:129] = Wv_b[128 * c + 64:128 * c + 128]
        bv_pad[0, 129] = 1.0
        im = {
            "xk_t": xk, "xq_t": xq, "xv_t": xv,
            "wk_h": wk_c, "wq_h": wq_c, "wv_p": wv_pad.astype(bf),
            "wo_h": np.ascontiguousarray(Wo_w.T[rows]).astype(bf),
            "bk_h": np.ascontiguousarray(
                Wk_b[rows].reshape(128, 1)).astype(np.float32),
            "bq_h": np.ascontiguousarray(
                Wq_b[rows].reshape(128, 1)).astype(np.float32),
            "bv_p": bv_pad, "ones1": ones1,
            "trimask": tri.astype(bf),
        }
        if general:
            im["maskb"] = maskb_arr
        in_maps.append(im)

    _BUILT["last_maps"] = in_maps
    res = run_bass_kernel_spmd(nc, in_maps, core_ids=list(range(NCORES)))
    _BUILT["last_result"] = res
    _BUILT["nc"] = nc

    acc = np.zeros((2 * S, D), np.float32)
    for c in range(NCORES):
        acc += np.asarray(res.results[c]["outp"], dtype=np.float32)
    acc += Wo_b[None, :].astype(np.float32)
    outf = np.empty((B, S, D), np.float32)
    outf[0] = acc[0:S]
    outf[1] = acc[S:2 * S]
    return outf


# revision 51
# speedup vs baseline: 1.2295x; 1.2295x over previous
"""Multi-headed causal attention (B=2, S=2048, D=1024, H=16, DK=DV=64) on 8
Trainium2 NeuronCores.

Sharding: HEAD-parallel, zero-communication. Core c owns heads {2c, 2c+1}
for BOTH batches: it projects K/Q/V for its two heads only (no redundant
FLOPs), runs their full causal attention, and output-projects ALL 4096
queries against its two heads' Wo rows, emitting a bf16 PARTIAL output.
The host sums the 8 partials and adds the output bias -- collectives in
this stack act as full program barriers (~30us each), so finishing on
the host is cheaper. Head-sharding makes the fine-causal tile structure
(only kt<=t score tiles) IDENTICAL on every core, as SPMD requires.

All matmul operands are bf16 (full PE rate at any free-dim). The PE is
HAM-clock-gated: idle gaps drop it to 1.2GHz, so emission keeps the PE
dense: b1's projections fill attention(b0) banks, b0's output projection
fills attention(b1) banks (deadline+pace scheduler). Later attention
phases exp over two PSUM banks per ACT instruction; causal masks run on
the idle Pool engine. Softmax: no max-subtraction; denominators via an
all-ones V column; reciprocal = fast-approx DVE op on the ACT-shifted
denominator row, replicated by a K=1 f32r matmul, applied in-place.
"""

import numpy as np

B, S, D, H, DK = 2, 2048, 1024, 16, 64
NCORES = 8
NT = S // 128
NBANKS = 34

_BUILT = {}


def _build_nc(general_mask):
    import concourse.bacc as bacc
    import concourse.mybir as mybir
    from concourse import tile
    from contextlib import ExitStack

    f32 = mybir.dt.float32
    f32r = mybir.dt.float32r
    bf16 = mybir.dt.bfloat16
    AF = mybir.ActivationFunctionType
    ALU = mybir.AluOpType

    nc = bacc.Bacc("TRN2", target_bir_lowering=False, debug=False,
                   num_devices=NCORES)

    xk_t = nc.declare_dram_parameter("xk_t", [2 * D, S], bf16, isOutput=False)
    xq_t = nc.declare_dram_parameter("xq_t", [2 * D, S], bf16, isOutput=False)
    xv_t = nc.declare_dram_parameter("xv_t", [2 * D, S], bf16, isOutput=False)
    wk_h = nc.declare_dram_parameter("wk_h", [D, 128], bf16, isOutput=False)
    wq_h = nc.declare_dram_parameter("wq_h", [D, 128], bf16, isOutput=False)
    wv_p = nc.declare_dram_parameter("wv_p", [D, 130], bf16, isOutput=False)
    wo_h = nc.declare_dram_parameter("wo_h", [128, D], bf16, isOutput=False)
    bk_h = nc.declare_dram_parameter("bk_h", [128, 1], f32, isOutput=False)
    bq_h = nc.declare_dram_parameter("bq_h", [128, 1], f32, isOutput=False)
    bv_p = nc.declare_dram_parameter("bv_p", [1, 130], f32, isOutput=False)
    ones1 = nc.declare_dram_parameter("ones1", [1, 128], f32r, isOutput=False)
    trimask = nc.declare_dram_parameter("trimask", [128, 128], bf16,
                                        isOutput=False)
    if general_mask:
        maskb = nc.declare_dram_parameter(
            "maskb", [2 * NBANKS * 128, 512], bf16, isOutput=False)
    outp = nc.declare_dram_parameter("outp", [2 * S, D], bf16, isOutput=True)

    with tile.TileContext(nc) as tc:
        with ExitStack() as ctx:
            persist = ctx.enter_context(tc.tile_pool(name="persist", bufs=1))

            wk_sb = [persist.tile([128, 128], bf16, name=f"wk{i}",
                                  tag=f"wk{i}") for i in range(8)]
            wq_sb = [persist.tile([128, 128], bf16, name=f"wq{i}",
                                  tag=f"wq{i}") for i in range(8)]
            wv_sb = [persist.tile([128, 130], bf16, name=f"wv{i}",
                                  tag=f"wv{i}") for i in range(8)]
            wo_sb = persist.tile([128, D], bf16, name="wo", tag="wo")
            bk_sb = persist.tile([128, 1], f32, name="bk", tag="bk")
            bq_sb = persist.tile([128, 1], f32, name="bq", tag="bq")
            tri_sb = persist.tile([128, 128], bf16, name="tri", tag="tri")
            bvr_sb = persist.tile([1, 130], f32, name="bvr", tag="bvr")
            ones_sb = persist.tile([1, 128], f32r, name="ones", tag="ones")
            bv_rep = persist.tile([128, 130], f32, name="bvrep", tag="bvrep")
            kT = [persist.tile([128, S], bf16, name=f"kT{b}", tag=f"kT{b}")
                  for b in range(B)]
            qT = [persist.tile([128, S], bf16, name=f"qT{b}", tag=f"qT{b}")
                  for b in range(B)]
            v_sb = [[persist.tile([128, 130], bf16, name=f"v{b}_{st}",
                                  tag=f"v{b}_{st}") for st in range(NT)]
                    for b in range(B)]
            navTh = [[persist.tile([64, S], bf16, name=f"nav{b}_{hh}",
                                   tag=f"nav{b}_{hh}") for hh in range(2)]
                     for b in range(B)]
            navT2 = [persist.tile([128, S], bf16, name=f"nv2_{b}",
                                  tag=f"nv2_{b}") for b in range(B)]

            xs = ctx.enter_context(tc.tile_pool(name="xs", bufs=2))
            amp = ctx.enter_context(tc.tile_pool(name="amp", bufs=3))
            nrm = ctx.enter_context(tc.tile_pool(name="nrm", bufs=2))
            fop = ctx.enter_context(tc.tile_pool(name="fop", bufs=3))
            avp = ctx.enter_context(tc.tile_pool(name="avp", bufs=2,
                                                 space="PSUM"))
            repp = ctx.enter_context(tc.tile_pool(name="repp", bufs=1,
                                                  space="PSUM"))
            if general_mask:
                mbp = ctx.enter_context(tc.tile_pool(name="mbp", bufs=4))

            # ---- P0: K-path loads first so the PE starts ASAP ----
            for i in range(8):
                nc.sync.dma_start(wk_sb[i][:], wk_h[128 * i:128 * (i + 1), :])
            nc.sync.dma_start(bk_sb[:], bk_h[:])

            def load_x_halves(param, b):
                tiles = [xs.tile([128, S], bf16, name=f"x{kp}", tag=f"x{kp}")
                         for kp in range(8)]
                for h in range(2):
                    for kp in range(8):
                        eng = nc.sync if kp % 2 == 0 else nc.scalar
                        eng.dma_start(
                            tiles[kp][:, 1024 * h:1024 * (h + 1)],
                            param[D * b + 128 * kp:D * b + 128 * (kp + 1),
                                  1024 * h:1024 * (h + 1)])
                return tiles

            pools = {}

            def proj_kq_unit(x, w_sb, bias_sb, dst, sc):
                ps = pools["pp"].tile([128, 512], f32, name="pp", tag="pp")
                for kp in range(8):
                    nc.tensor.matmul(ps[:], w_sb[kp][:],
                                     x[kp][:, 512 * sc:512 * (sc + 1)],
                                     start=(kp == 0), stop=(kp == 7))
                nc.vector.tensor_scalar_add(
                    dst[:, 512 * sc:512 * (sc + 1)], ps[:], bias_sb[:])

            def proj_v_unit(x, b, st):
                ps = pools["pp"].tile([128, 512], f32, name="pp", tag="pp")
                for kp in range(8):
                    nc.tensor.matmul(ps[:, 0:130],
                                     x[kp][:, 128 * st:128 * (st + 1)],
                                     wv_sb[kp][:],
                                     start=(kp == 0), stop=(kp == 7))
                nc.vector.tensor_tensor(v_sb[b][st][:], ps[:, 0:130],
                                        bv_rep[:], ALU.add)

            def op_unit(b, qt, oc, u):
                # partial output projection: this core's 2 heads only,
                # K=128, one matmul; host sums partials across cores
                ps = pools["scp2"].tile([128, 1024], f32, name="sc",
                                        tag="sc")
                nc.tensor.matmul(ps[:, 0:512],
                                 navT2[b][:, 128 * qt:128 * (qt + 1)],
                                 wo_sb[:, 512 * oc:512 * (oc + 1)],
                                 start=True, stop=True)
                fo = fop.tile([128, 512], bf16, name="fo", tag="fo")
                nc.vector.tensor_copy(fo[:], ps[:, 0:512])
                nc.sync.dma_start(
                    outp[S * b + 128 * qt:S * b + 128 * (qt + 1),
                         512 * oc:512 * (oc + 1)], fo[:])

            def norm_stage1(b, hh, av, T):
                dg0 = nrm.tile([1, 512], f32, name="dg0", tag="dg0")
                nc.scalar.copy(dg0[:], av[64:65, :])
                dg = nrm.tile([1, 512], f32, name="dg", tag="dg")
                nc.vector.reciprocal_approx_fast(dg[:], dg0[:])
                dgr = nrm.tile([1, 512], f32r, name="dgr", tag="dgr")
                nc.vector.tensor_scalar_add(dgr[:], dg[:], 0.0)
                nc.vector.tensor_copy(
                    navTh[b][hh][:, 512 * T:512 * (T + 1)], av[0:64, :])
                return dgr

            def norm_stage2(b, hh, dgr, T):
                rp = repp.tile([64, 512], f32, name="rp", tag="rp")
                nc.tensor.matmul(rp[:], ones_sb[:, 0:64], dgr[:],
                                 start=True, stop=True)
                sl = navTh[b][hh][:, 512 * T:512 * (T + 1)]
                nc.vector.tensor_tensor(sl, sl, rp[:], ALU.mult)
                nc.sync.dma_start(
                    navT2[b][64 * hh:64 * (hh + 1),
                             512 * T:512 * (T + 1)], sl)

            stream = [(t, kt) for t in range(NT) for kt in range(t + 1)]
            banks = [stream[i:i + 4] for i in range(0, len(stream), 4)]

            fill_units = []
            fill_state = {"idx": 0}

            def pump(g, g_lo=0, g_hi=0):
                # strictly deadline-driven: deadlines both spread the filler
                # and guarantee producers are emitted before consumers
                n = len(fill_units)
                while fill_state["idx"] < n:
                    i = fill_state["idx"]
                    if fill_units[i][0] > g:
                        break
                    fill_units[i][1]()
                    fill_state["idx"] += 1

            def attention(b, hh, g0, g_lo, g_hi, wide=False):
                r0 = 64 * hh
                step = 2 if wide else 1
                av = None
                pending = None
                for bi in range(0, NBANKS, step):
                    pump(g0 + bi, g_lo, g_hi)
                    flat = [tk for bk in banks[bi:bi + step] for tk in bk]
                    width = 128 * len(flat)
                    sc = pools["scp2" if wide else "scp"].tile(
                        [128, 1024 if wide else 512], f32, name="sc",
                        tag="sc")
                    for s, (t, kt) in enumerate(flat):
                        nc.tensor.matmul(
                            sc[:, 128 * s:128 * (s + 1)],
                            kT[b][r0:r0 + 64, 128 * kt:128 * (kt + 1)],
                            qT[b][r0:r0 + 64, 128 * t:128 * (t + 1)],
                            start=True, stop=True)
                    am = amp.tile([128, 1024 if wide else 512], bf16,
                                  name="am", tag="am2" if wide else "am")
                    nc.scalar.activation(am[:, 0:width], sc[:, 0:width],
                                         AF.Exp, scale=0.125)
                    if general_mask:
                        for k2 in range((len(flat) + 3) // 4):
                            mb = mbp.tile([128, 512], bf16, name="mb",
                                          tag="mb")
                            r = (b * NBANKS + bi + k2) * 128
                            nc.sync.dma_start(mb[:], maskb[r:r + 128, :])
                            nc.vector.tensor_tensor(
                                am[:, 512 * k2:512 * (k2 + 1)],
                                am[:, 512 * k2:512 * (k2 + 1)],
                                mb[:], ALU.mult)
                    else:
                        for s, (t, kt) in enumerate(flat):
                            if t == kt:
                                nc.gpsimd.tensor_tensor(
                                    am[:, 128 * s:128 * (s + 1)],
                                    am[:, 128 * s:128 * (s + 1)],
                                    tri_sb[:], ALU.mult)
                    for s, (t, kt) in enumerate(flat):
                        if kt == 0 and t % 4 == 0:
                            av = avp.tile([65, 512], f32, name="av",
                                          tag="av")
                        nc.tensor.matmul(
                            av[:, 128 * (t % 4):128 * (t % 4 + 1)],
                            v_sb[b][kt][:, 65 * hh:65 * (hh + 1)],
                            am[:, 128 * s:128 * (s + 1)],
                            start=(kt == 0), stop=(kt == t))
                        if kt == t and t % 4 == 3:
                            if pending is not None:
                                norm_stage2(b, hh, pending[0], pending[1])
                            dgr = norm_stage1(b, hh, av, t // 4)
                            pending = (dgr, t // 4)
                if pending is not None:
                    norm_stage2(b, hh, pending[0], pending[1])

            # ---- P0 continued + b0 K/Q projection ----
            xk0 = load_x_halves(xk_t, 0)
            for i in range(8):
                nc.scalar.dma_start(wq_sb[i][:],
                                    wq_h[128 * i:128 * (i + 1), :])
                nc.gpsimd.dma_start(wv_sb[i][:],
                                    wv_p[128 * i:128 * (i + 1), :])
            nc.gpsimd.dma_start(wo_sb[:], wo_h[:])
            nc.gpsimd.dma_start(bq_sb[:], bq_h[:])
            nc.gpsimd.dma_start(tri_sb[:], trimask[:])
            nc.gpsimd.dma_start(bvr_sb[:], bv_p[:])
            nc.gpsimd.dma_start(ones_sb[:], ones1[:])
            nc.gpsimd.partition_broadcast(bv_rep[:], bvr_sb[:])

            with tc.tile_pool(name="pp", bufs=2, space="PSUM") as pp, \
                 tc.tile_pool(name="scp", bufs=3, space="PSUM") as scp:
                pools["pp"] = pp
                pools["scp"] = scp

                for sc_i in range(4):
                    proj_kq_unit(xk0, wk_sb, bk_sb, kT[0], sc_i)
                xq0 = load_x_halves(xq_t, 0)
                for sc_i in range(4):
                    proj_kq_unit(xq0, wq_sb, bq_sb, qT[0], sc_i)
                xv0 = load_x_halves(xv_t, 0)

                def dl_diag(st):
                    return (st * (st + 3) // 2) // 4

                for st in range(NT):
                    fill_units.append(
                        (dl_diag(st),
                         (lambda st=st: proj_v_unit(xv0, 0, st))))
                # b1 projections spread over (0,0)+(0,1); consumers start
                # at g=68 so only the xs-ring order matters: k before q
                xk1_t = {}
                fill_units.append(
                    (36, lambda: xk1_t.update(x=load_x_halves(xk_t, 1))))
                for sc_i in range(4):
                    fill_units.append(
                        (38 + 2 * sc_i,
                         (lambda sc_i=sc_i: proj_kq_unit(
                             xk1_t["x"], wk_sb, bk_sb, kT[1], sc_i))))
                xv1_t = {}
                fill_units.append(
                    (46, lambda: xv1_t.update(x=load_x_halves(xv_t, 1))))
                for st in range(NT):
                    fill_units.append(
                        (47 + st,
                         (lambda st=st: proj_v_unit(xv1_t["x"], 1, st))))
                xq1_t = {}
                fill_units.append(
                    (48, lambda: xq1_t.update(x=load_x_halves(xq_t, 1))))
                for sc_i in range(4):
                    fill_units.append(
                        (50 + 4 * sc_i,
                         (lambda sc_i=sc_i: proj_kq_unit(
                             xq1_t["x"], wq_sb, bq_sb, qT[1], sc_i))))
                fill_units.sort(key=lambda u: u[0])

                attention(0, 0, 0, 0, 68)
                attention(0, 1, 34, 0, 68)
                pump(68, 0, 68)

            with tc.tile_pool(name="scp2", bufs=2, space="PSUM") as scp2:
                pools["scp2"] = scp2

                # b0 output-projection partials fill attention(b1)
                fill_units.clear()
                fill_state["idx"] = 0
                for u in range(32):
                    qt, oc = u // 2, u % 2
                    fill_units.append(
                        (70 + 2 * u,
                         (lambda qt=qt, oc=oc, u=u: op_unit(0, qt, oc, u))))

                attention(1, 0, 68, 68, 136, wide=True)
                # b1 output projection: navT2[1] block T is complete once
                # both (1,0) and (1,1) have normalized it -> pump early
                # units during (1,1), drain the rest as the tail
                for u in range(32):
                    qt, oc = u // 2, u % 2
                    T = qt // 4
                    # navT2[1] block T's lagged stage2 is emitted by the end
                    # of block T+1 (~bank 8*(T+2) of (1,1)); T>=2 drains in
                    # the tail
                    dl = 137 if T >= 2 else (112 + 10 * T + (u % 8))
                    fill_units.append(
                        (dl,
                         (lambda qt=qt, oc=oc, u=u: op_unit(1, qt, oc, u))))
                attention(1, 1, 102, 68, 136, wide=True)
                pump(137)

    nc.compile()
    return nc


def kernel(V, K, Q, padding_mask, Wv_w, Wv_b, Wk_w, Wk_b, Wq_w, Wq_b,
           Wo_w, Wo_b):
    import ml_dtypes
    from concourse.bass_utils import run_bass_kernel_spmd
    bf = ml_dtypes.bfloat16

    V = np.asarray(V, np.float32)
    K = np.asarray(K, np.float32)
    Q = np.asarray(Q, np.float32)
    pm = np.asarray(padding_mask)
    Wv_w = np.asarray(Wv_w, np.float32)
    Wv_b = np.asarray(Wv_b, np.float32)
    Wk_w = np.asarray(Wk_w, np.float32)
    Wk_b = np.asarray(Wk_b, np.float32)
    Wq_w = np.asarray(Wq_w, np.float32)
    Wq_b = np.asarray(Wq_b, np.float32)
    Wo_w = np.asarray(Wo_w, np.float32)
    Wo_b = np.asarray(Wo_b, np.float32)

    general = not bool((pm != 0).all())
    key = "gen" if general else "fast"
    if key not in _BUILT:
        _BUILT[key] = _build_nc(general)
    nc = _BUILT[key]

    xk = np.concatenate(
        [np.ascontiguousarray(K[b].T) for b in range(B)], 0).astype(bf)
    xq = np.concatenate(
        [np.ascontiguousarray(Q[b].T) for b in range(B)], 0).astype(bf)
    xv = np.concatenate(
        [np.ascontiguousarray(V[b].T) for b in range(B)], 0).astype(bf)
    ones1 = np.ones((1, 128), np.float32)
    tri = (np.arange(128)[:, None] <= np.arange(128)[None, :])

    maskb_arr = None
    if general:
        stream = [(t, kt) for t in range(NT) for kt in range(t + 1)]
        bank_list = [stream[i:i + 4] for i in range(0, len(stream), 4)]
        maskb_arr = np.zeros((2 * NBANKS * 128, 512), np.float32)
        for b in range(B):
            keymask = (pm[b] != 0).astype(np.float32)
            for bi, bank in enumerate(bank_list):
                blk = np.zeros((128, 512), np.float32)
                for s, (t, kt) in enumerate(bank):
                    m = np.ones((128, 128), np.float32) if kt < t \
                        else tri.astype(np.float32)
                    blk[:, 128 * s:128 * (s + 1)] = (
                        m * keymask[128 * kt:128 * (kt + 1)][:, None])
                maskb_arr[(b * NBANKS + bi) * 128:
                          (b * NBANKS + bi + 1) * 128] = blk
        maskb_arr = maskb_arr.astype(bf)

    in_maps = []
    for c in range(NCORES):
        rows = slice(128 * c, 128 * (c + 1))
        wk_c = np.ascontiguousarray(Wk_w[rows].T).astype(bf)
        wq_c = np.ascontiguousarray(Wq_w[rows].T).astype(bf)
        wv_c = np.ascontiguousarray(Wv_w[rows].T)
        wv_pad = np.zeros((D, 130), np.float32)
        wv_pad[:, 0:64] = wv_c[:, 0:64]
        wv_pad[:, 65:129] = wv_c[:, 64:128]
        bv_pad = np.zeros((1, 130), np.float32)
        bv_pad[0, 0:64] = Wv_b[128 * c:128 * c + 64]
        bv_pad[0, 64] = 1.0
        bv_pad[0, 65:129] = Wv_b[128 * c + 64:128 * c + 128]
        bv_pad[0, 129] = 1.0
        im = {
            "xk_t": xk, "xq_t": xq, "xv_t": xv,
            "wk_h": wk_c, "wq_h": wq_c, "wv_p": wv_pad.astype(bf),
            "wo_h": np.ascontiguousarray(Wo_w.T[rows]).astype(bf),
            "bk_h": np.ascontiguousarray(
                Wk_b[rows].reshape(128, 1)).astype(np.float32),
            "bq_h": np.ascontiguousarray(
                Wq_b[rows].reshape(128, 1)).astype(np.float32),
            "bv_p": bv_pad, "ones1": ones1,
            "trimask": tri.astype(bf),
        }
        if general:
            im["maskb"] = maskb_arr
        in_maps.append(im)

    _BUILT["last_maps"] = in_maps
    res = run_bass_kernel_spmd(nc, in_maps, core_ids=list(range(NCORES)))
    _BUILT["last_result"] = res
    _BUILT["nc"] = nc

    acc = np.zeros((2 * S, D), np.float32)
    for c in range(NCORES):
        acc += np.asarray(res.results[c]["outp"], dtype=np.float32)
    acc += Wo_b[None, :].astype(np.float32)
    outf = np.empty((B, S, D), np.float32)
    outf[0] = acc[0:S]
    outf[1] = acc[S:2 * S]
    return outf
